# revision 67
# baseline (speedup 1.0000x reference)
"""Trainium2 (8 NeuronCores) kernel for batched 'general' attention:

    energy  = h_s @ W_in^T                     [B,S,D]
    scores  = h_t @ energy^T  (masked, clamped)[B,T,S]
    probs   = softmax(scores, axis=-1)
    context = probs @ h_s                      [B,T,D]
    returns (context, probs)

Strategy:
  * Data-parallel over batch: 2 batches per core on 8 cores (no collectives).
  * Algebraic rewrite: scores = (h_t @ W_in) @ h_s^T  (saves the S*D*D matmul).
  * Mask sparsity: ~half the source positions are masked out (prob exactly 0
    in the reference).  The host compacts h_s to the unmasked columns, the
    device computes attention over the compact S, and the host scatters the
    probabilities back into the full [T,S] output (zeros elsewhere).
  * float32r (tf32-style) matmuls at 4x fp32 PE throughput; inputs are
    RNE-rounded to tf32 on the host so the f32r reads are well-defined.
  * Context is accumulated from exp(scores - max) and scaled by 1/denom at
    the PSUM->SBUF copy, so the PE transposes don't wait on normalization.
  * Dense fallback (mask mostly ones): additive -1e11 bias folded into the
    PSUM accumulation via a K=1 matmul, clamp fused into the PSUM->SBUF copy.

Self-contained: hardcodes the problem shapes; only needs the concourse repo
(/opt/trn_rl_repo) for bass.
"""

import os
import sys

import numpy as np

for _p in ("/opt/trn_rl_repo", os.path.expanduser("~/.axon_site/_ro/trn_rl_repo")):
    if os.path.isdir(_p) and _p not in sys.path:
        sys.path.insert(0, _p)

import concourse.bacc as bacc
import concourse.mybir as mybir
from concourse.bass_utils import run_bass_kernel_spmd
from concourse.masks import make_identity
from concourse.tile import TileContext, add_dep_helper

B, T, S, D = 16, 1024, 4096, 1024
N_CORES = 8
NB = B // N_CORES  # batches per core
P = 128
TI_FULL = T // P

F32 = mybir.dt.float32
F32R = mybir.dt.float32r
BF16 = mybir.dt.bfloat16


def _dt(env, default):
    v = os.environ.get(env, default)
    return {"f32": F32, "f32r": F32R, "bf16": BF16}[v]


# Matmul compute dtypes (f32r = tf32 inputs, fp32 accumulate, 4x faster).
DT_Q = _dt("ATT_DT_Q", "f32r")  # q = h_t @ W
DT_SC = _dt("ATT_DT_SC", "f32r")  # scores = q @ h_s^T
DT_CTX = _dt("ATT_DT_CTX", "bf16")  # context = probs @ h_s

MAX_SPARSE_S_PAD = 2560  # SBUF budget limit for the single-pass resident layout
SBUF_KB = 192  # per-partition budget enforced by the tile allocator


def round_tf32(x):
    """Round-to-nearest-even to a 10-bit mantissa (tf32), in float32 storage."""
    u = np.ascontiguousarray(x, np.float32).view(np.uint32)
    r = (u >> np.uint32(13)) & np.uint32(1)
    u = (u + np.uint32(0x0FFF) + r) & np.uint32(0xFFFFE000)
    return u.view(np.float32)


def _maybe_round(x, dt):
    if dt == F32R:
        return round_tf32(x)
    if dt == BF16:
        return np.ascontiguousarray(x).astype(mybir.dt.np(BF16))
    return np.ascontiguousarray(x, np.float32)


def build_kernel(s_pad, t_tiles_per_pass, use_bias, dt_q, dt_sc, dt_ctx):
    nc = bacc.Bacc(None, target_bir_lowering=False)
    KD = D // P  # contraction tiles over the feature dim
    TI = T // P  # t row-tiles
    SK = s_pad // P  # s tiles
    assert TI % t_tiles_per_pass == 0
    n_pass = TI // t_tiles_per_pass
    TB = t_tiles_per_pass
    single = n_pass == 1

    h_tT = nc.declare_dram_parameter("h_tT", [NB, D, T], dt_q, isOutput=False)
    w_in = nc.declare_dram_parameter("W", [D, D], dt_q, isOutput=False)
    h_sT = nc.declare_dram_parameter("h_sT", [NB, D, s_pad], dt_sc, isOutput=False)
    h_sN = nc.declare_dram_parameter("h_sN", [NB, s_pad, D], dt_ctx, isOutput=False)
    if use_bias:
        biasd = nc.declare_dram_parameter("bias", [NB, s_pad], F32, isOutput=False)
    probs = nc.declare_dram_parameter("probs", [NB, T, s_pad], F32, isOutput=True)
    ctx = nc.declare_dram_parameter("context", [NB, T, D], F32, isOutput=True)

    # Balanced s-chunks (all >=256 wide where possible) so no thin tail matmuls.
    n_chunks = -(-s_pad // 512)
    base = min(512, -(-(-(-s_pad // n_chunks)) // 64) * 64)
    s_chunks = []
    c = 0
    while c < s_pad:
        cw = min(base, s_pad - c)
        s_chunks.append((c, cw))
        c += cw

    # MM3 moving-operand chunk width (PSUM bank limit: 512 fp32 outputs).
    dc = 512
    hs_tile_kb = D * mybir.dt.size(dt_ctx) / 1024  # per-partition KB of one h_s tile

    # h_s-natural tiles prefetched (during phase B) vs loaded after phase B.
    if single:
        used = (
            0.5  # identity
            + KD * D * 4 / 1024  # w_sb
            + KD * T * 4 / 1024  # qT
            + TB * s_pad * 4 / 1024  # scmat
            + 2 * KD * base * 4 / 1024  # hsT chunks (bufs=2)
            + 4  # slack
        )
        a_tiles = max(0, min(SK, int((SBUF_KB - used) // hs_tile_kb)))
    else:
        a_tiles = 0

    with TileContext(nc) as tc:
        with tc.tile_pool(name="const", bufs=1) as const_pool:
            # Dependency-free warmup matmuls: get the PE sequencer into the
            # kernel body and the HAM clock-gate to full rate while the first
            # inputs stream in.
            warm = const_pool.tile([P, 512], F32)
            nc.vector.memset(warm, 0.0)
            with tc.tile_pool(name="pswarm", bufs=1, space="PSUM") as pswarm:
                wps = pswarm.tile([P, 512], F32)
                for _ in range(3):
                    nc.tensor.matmul(
                        wps[0:16, :], lhsT=warm[:, 0:16], rhs=warm, start=True, stop=True
                    )
            ident = const_pool.tile([P, P], F32)
            make_identity(nc, ident)
            use_bf_tr = single and dt_ctx == BF16
            if use_bf_tr:
                ident_bf = const_pool.tile([P, P], BF16)
                make_identity(nc, ident_bf)
            if use_bias:
                ones_t = const_pool.tile([1, P], F32)
                nc.vector.memset(ones_t, 1.0)
            w_sb = const_pool.tile([P, KD, D], dt_q)
            w_src = w_in[:, :].rearrange("(k p) d -> p k d", p=P)

            for b in range(NB):
                qT_pool = tc.alloc_tile_pool(name="qTp", bufs=1, side="right")
                if True:
                    # qT holds (h_t @ W)^T pre-rounded to the MM2 input dtype
                    qT = qT_pool.tile([P, KD, T], dt_sc)
                    # ---------------- Phase A: qT[d, t] = (h_t @ W)^T ----------------
                    with (
                        tc.tile_pool(name="phA", bufs=1, side="right") as pa,
                        tc.tile_pool(name="psA", bufs=2, space="PSUM") as psA,
                    ):
                        ht_sb = pa.tile([P, KD, T], dt_q)
                        ht_src = h_tT[b, :, :].rearrange("(k p) t -> p k t", p=P)
                        deferred = []
                        t_chunks = [(0, 256), (256, 256)] + [
                            (n0, 512) for n0 in range(512, T, 512)
                        ]
                        nc.sync.dma_start(
                            out=ht_sb[:, :, 0:256], in_=ht_src[:, :, 0:256]
                        )
                        if b == 0:
                            for j in range(KD):
                                d = nc.sync.dma_start(
                                    out=w_sb[:, :, j * P : (j + 1) * P],
                                    in_=w_src[:, :, j * P : (j + 1) * P],
                                )
                                if j >= 1:
                                    deferred.append(d)
                        for n0, nw in t_chunks[1:]:
                            deferred.append(
                                nc.sync.dma_start(
                                    out=ht_sb[:, :, n0 : n0 + nw],
                                    in_=ht_src[:, :, n0 : n0 + nw],
                                )
                            )
                        first_mm = None
                        for n0, nw in t_chunks:
                            for j in range(KD):
                                psa = psA.tile([P, 512], F32, tag="mmq")
                                for k in range(KD):
                                    mm = nc.tensor.matmul(
                                        psa[:, :nw],
                                        lhsT=w_sb[:, k, j * P : (j + 1) * P],
                                        rhs=ht_sb[:, k, n0 : n0 + nw],
                                        start=(k == 0),
                                        stop=(k == KD - 1),
                                    )
                                    if first_mm is None:
                                        first_mm = mm
                                nc.scalar.copy(
                                    out=qT[:, j, n0 : n0 + nw], in_=psa[:, :nw]
                                )
                        # Let the first matmul's inputs win the DMA bandwidth:
                        # the bulk of W/h_t loads only needs to land during MM-q.
                        for d in deferred:
                            add_dep_helper(
                                d.ins, first_mm.ins, reason="defer bulk input DMA"
                            )
                        gate_mm = first_mm

                    for ip in range(n_pass):
                        tlo = ip * TB
                        with tc.tile_pool(name="scores", bufs=1) as sc_pool:
                            scmat = sc_pool.tile([P, TB, s_pad], F32)
                            negmax = [
                                sc_pool.tile([P, 1], F32, tag=f"negmax{i}", name=f"negmax{i}")
                                for i in range(TB)
                            ]
                            denom = [
                                sc_pool.tile([P, 1], F32, tag=f"denom{i}", name=f"denom{i}")
                                for i in range(TB)
                            ]
                            recip = [
                                sc_pool.tile([P, 1], F32, tag=f"recip{i}", name=f"recip{i}")
                                for i in range(TB)
                            ]
                            den2 = [
                                sc_pool.tile([P, 1], F32, tag=f"den2{i}", name=f"den2{i}")
                                for i in range(TB)
                            ]
                            nchk = len(s_chunks)
                            pmax = [
                                sc_pool.tile([P, nchk], F32, tag=f"pmax{i}", name=f"pmax{i}")
                                for i in range(TB)
                            ]
                            with tc.tile_pool(name="hsa", bufs=1) as hsa_pool:
                                if a_tiles:
                                    hs_a = hsa_pool.tile([P, a_tiles, D], dt_ctx)
                                    for k in range(a_tiles):
                                        d = nc.sync.dma_start(
                                            out=hs_a[:, k, :],
                                            in_=h_sN[b, k * P : (k + 1) * P, :],
                                        )
                                        add_dep_helper(
                                            d.ins, gate_mm.ins, reason="defer hs_a DMA"
                                        )
                                # ---- Phase B: scores = qT^T @ h_sT (+bias), softmax ----
                                with (
                                    tc.tile_pool(name="phB", bufs=2) as pb,
                                    tc.tile_pool(name="psB", bufs=6, space="PSUM") as psB,
                                    tc.tile_pool(name="biasp", bufs=1) as bp,
                                ):
                                    if use_bias:
                                        bias_sb = bp.tile([1, s_pad], F32)
                                        nc.sync.dma_start(
                                            out=bias_sb, in_=biasd[b : b + 1, :]
                                        )
                                    for ci, (c0, cw) in enumerate(s_chunks):
                                        hsT_c = pb.tile([P, KD, base], dt_sc, tag="hsT")
                                        d = nc.sync.dma_start(
                                            out=hsT_c[:, :, :cw],
                                            in_=h_sT[b, :, c0 : c0 + cw].rearrange(
                                                "(k p) s -> p k s", p=P
                                            ),
                                        )
                                        if ci < 2:
                                            add_dep_helper(
                                                d.ins, gate_mm.ins, reason="defer hsT DMA"
                                            )
                                        for i in range(TB):
                                            ti = tlo + i
                                            psb = psB.tile([P, 512], F32, tag="mm2")
                                            if use_bias:
                                                nc.tensor.matmul(
                                                    psb[:, :cw],
                                                    lhsT=ones_t,
                                                    rhs=bias_sb[:, c0 : c0 + cw],
                                                    start=True,
                                                    stop=False,
                                                )
                                            for k in range(KD):
                                                nc.tensor.matmul(
                                                    psb[:, :cw],
                                                    lhsT=qT[:, k, ti * P : (ti + 1) * P],
                                                    rhs=hsT_c[:, k, :cw],
                                                    start=(k == 0 and not use_bias),
                                                    stop=(k == KD - 1),
                                                )
                                            if use_bias:
                                                nc.vector.tensor_scalar_max(
                                                    out=scmat[:, i, c0 : c0 + cw],
                                                    in0=psb[:, :cw],
                                                    scalar1=-1e10,
                                                )
                                            else:
                                                nc.vector.tensor_copy(
                                                    out=scmat[:, i, c0 : c0 + cw],
                                                    in_=psb[:, :cw],
                                                )
                                            nc.vector.tensor_reduce(
                                                out=pmax[i][:, ci : ci + 1],
                                                in_=scmat[:, i, c0 : c0 + cw],
                                                axis=mybir.AxisListType.X,
                                                op=mybir.AluOpType.max,
                                            )
                                # softmax: exp in place; normalization of the probs
                                # output happens after the transposes read exp().
                                sp = max(P, (SK // 2) * P)
                                for i in range(TB):
                                    nc.vector.tensor_reduce(
                                        out=negmax[i],
                                        in_=pmax[i],
                                        axis=mybir.AxisListType.X,
                                        op=mybir.AluOpType.max,
                                        negate=True,
                                    )
                                    nc.scalar.activation(
                                        out=scmat[:, i, 0:sp],
                                        in_=scmat[:, i, 0:sp],
                                        func=mybir.ActivationFunctionType.Exp,
                                        bias=negmax[i],
                                        scale=1.0,
                                        accum_out=denom[i],
                                    )
                                    if sp < s_pad:
                                        nc.scalar.activation(
                                            out=scmat[:, i, sp:s_pad],
                                            in_=scmat[:, i, sp:s_pad],
                                            func=mybir.ActivationFunctionType.Exp,
                                            bias=negmax[i],
                                            scale=1.0,
                                            accum_out=den2[i],
                                        )
                                        nc.vector.tensor_add(
                                            out=denom[i], in0=denom[i], in1=den2[i]
                                        )
                                    nc.vector.reciprocal(out=recip[i], in_=denom[i])
                                # ---------- Phase C: context = probs @ h_s ----------
                                if ip == n_pass - 1:
                                    qT_pool.release()
                                sg = max(1, SK - a_tiles) if single else 6
                                groups = [
                                    (g, min(sg, SK - g)) for g in range(a_tiles, SK, sg)
                                ]
                                with (
                                    tc.tile_pool(name="phC", bufs=(1 if single else 2)) as pc,
                                    tc.tile_pool(name="prTp", bufs=2) as prp,
                                    tc.tile_pool(name="ctxp", bufs=(2 if single else 1)) as cxp,
                                    tc.tile_pool(name="scbfp", bufs=1) as sbfp,
                                    tc.tile_pool(name="psC", bufs=3, space="PSUM") as psC,
                                    tc.tile_pool(name="psT", bufs=2, space="PSUM") as psT,
                                ):
                                    if use_bf_tr:
                                        # bf16 copy of exp(scores): prT is bf16
                                        # anyway, so rounding before the PE
                                        # transpose halves its cycle cost with
                                        # bit-identical MM3 inputs.
                                        scbf = sbfp.tile([P, TB, s_pad], BF16)
                                    if not single:
                                        ctx_acc = [
                                            cxp.tile(
                                                [P, D],
                                                F32,
                                                tag=f"ctxacc{i}",
                                                name=f"ctxacc{i}",
                                            )
                                            for i in range(TB)
                                        ]
                                    hs_bt = {}

                                    def load_group(g0, gw):
                                        hs_gr = pc.tile(
                                            [P, sg, D], dt_ctx, tag="hsgr", name="hsgr"
                                        )
                                        for k0 in range(0, gw, 5):
                                            kw = min(5, gw - k0)
                                            nc.sync.dma_start(
                                                out=hs_gr[:, k0 : k0 + kw, :],
                                                in_=h_sN[
                                                    b,
                                                    (g0 + k0) * P : (g0 + k0 + kw) * P,
                                                    :,
                                                ].rearrange("(k p) d -> p k d", p=P),
                                            )
                                        for k in range(gw):
                                            hs_bt[g0 + k] = hs_gr[:, k, :]

                                    if single:
                                        for g0, gw in groups:
                                            load_group(g0, gw)
                                        groups = [(0, SK)]
                                        if a_tiles:
                                            for k in range(a_tiles):
                                                hs_bt[k] = hs_a[:, k, :]
                                    for g0, gw in groups:
                                        if not single:
                                            load_group(g0, gw)
                                        for i in range(TB):
                                            prT = prp.tile(
                                                [P, sg if not single else SK, P],
                                                dt_ctx,
                                                tag="prT",
                                                name="prT",
                                            )
                                            if use_bf_tr:
                                                # split at the exp-half boundary
                                                # so the first transposes start
                                                # while the second exp half runs
                                                nc.vector.tensor_copy(
                                                    out=scbf[:, i, 0:sp],
                                                    in_=scmat[:, i, 0:sp],
                                                )
                                                if sp < s_pad:
                                                    nc.vector.tensor_copy(
                                                        out=scbf[:, i, sp:s_pad],
                                                        in_=scmat[:, i, sp:s_pad],
                                                    )
                                            for k in range(gw):
                                                if use_bf_tr:
                                                    pst = psT.tile(
                                                        [P, P], BF16, tag="tr", name="pst"
                                                    )
                                                    nc.tensor.transpose(
                                                        pst,
                                                        in_=scbf[
                                                            :,
                                                            i,
                                                            (g0 + k) * P : (g0 + k + 1) * P,
                                                        ],
                                                        identity=ident_bf,
                                                    )
                                                else:
                                                    pst = psT.tile(
                                                        [P, P], F32, tag="tr", name="pst"
                                                    )
                                                    nc.tensor.transpose(
                                                        pst,
                                                        in_=scmat[
                                                            :,
                                                            i,
                                                            (g0 + k) * P : (g0 + k + 1) * P,
                                                        ],
                                                        identity=ident,
                                                    )
                                                nc.vector.tensor_copy(
                                                    out=prT[:, k, :], in_=pst
                                                )
                                            for d0 in range(0, D, dc):
                                                psc = psC.tile(
                                                    [P, dc], F32, tag="mm3", name="psc"
                                                )
                                                for k in range(gw):
                                                    nc.tensor.matmul(
                                                        psc,
                                                        lhsT=prT[:, k, :],
                                                        rhs=hs_bt[g0 + k][:, d0 : d0 + dc],
                                                        start=(k == 0),
                                                        stop=(k == gw - 1),
                                                    )
                                                if single:
                                                    if d0 == 0:
                                                        ctx_sb = cxp.tile(
                                                            [P, D],
                                                            F32,
                                                            tag="ctx",
                                                            name="ctx_sb",
                                                        )
                                                    nc.scalar.mul(
                                                        out=ctx_sb[:, d0 : d0 + dc],
                                                        in_=psc,
                                                        mul=recip[i],
                                                    )
                                                elif g0 == a_tiles:
                                                    nc.scalar.mul(
                                                        out=ctx_acc[i][:, d0 : d0 + dc],
                                                        in_=psc,
                                                        mul=recip[i],
                                                    )
                                                else:
                                                    nc.vector.scalar_tensor_tensor(
                                                        out=ctx_acc[i][:, d0 : d0 + dc],
                                                        in0=psc,
                                                        scalar=recip[i],
                                                        in1=ctx_acc[i][:, d0 : d0 + dc],
                                                        op0=mybir.AluOpType.mult,
                                                        op1=mybir.AluOpType.add,
                                                    )
                                            if single:
                                                nc.sync.dma_start(
                                                    out=ctx[
                                                        b,
                                                        (tlo + i) * P : (tlo + i + 1) * P,
                                                        :,
                                                    ],
                                                    in_=ctx_sb,
                                                )
                                                # normalize probs for row-tile i
                                                # right after its transposes read
                                                # the unnormalized exp values;
                                                # on ScalarE so the DVE cast
                                                # chain feeding MM3 stays clear
                                                nc.scalar.mul(
                                                    out=scmat[:, i, :],
                                                    in_=scmat[:, i, :],
                                                    mul=recip[i],
                                                )
                                                nc.sync.dma_start(
                                                    out=probs[
                                                        b,
                                                        (tlo + i) * P : (tlo + i + 1) * P,
                                                        :,
                                                    ],
                                                    in_=scmat[:, i, :],
                                                )
                                    if not single:
                                        for i in range(TB):
                                            nc.vector.tensor_scalar_mul(
                                                out=scmat[:, i, :],
                                                in0=scmat[:, i, :],
                                                scalar1=recip[i],
                                            )
                                            nc.sync.dma_start(
                                                out=probs[
                                                    b, (tlo + i) * P : (tlo + i + 1) * P, :
                                                ],
                                                in_=scmat[:, i, :],
                                            )
                                        for i in range(TB):
                                            nc.sync.dma_start(
                                                out=ctx[
                                                    b, (tlo + i) * P : (tlo + i + 1) * P, :
                                                ],
                                                in_=ctx_acc[i],
                                            )
    return nc


def _prepare(h_t, h_s, W_in, m_s):
    """Choose sparse/dense layout and build per-core input maps."""
    idx = [np.flatnonzero(m_s[b]) for b in range(B)]
    n = [len(ix) for ix in idx]
    n_max = max(n) if n else 0
    s_pad = max(P, ((n_max + P - 1) // P) * P)
    use_sparse = (
        s_pad <= 3584 and min(n) > 0 and os.environ.get("ATT_FORCE_DENSE") != "1"
    )
    if use_sparse:
        t_pp, use_bias = (TI_FULL if s_pad <= MAX_SPARSE_S_PAD else 4), False
    else:
        s_pad, t_pp, use_bias = S, 2, True

    W_q = _maybe_round(W_in, DT_Q)
    in_maps = []
    for c in range(N_CORES):
        bs = list(range(c * NB, (c + 1) * NB))
        h_tT_c = _maybe_round(h_t[bs].transpose(0, 2, 1), DT_Q)
        if use_sparse:
            hsN = np.zeros((NB, s_pad, D), np.float32)
            for j, b in enumerate(bs):
                hsN[j, : n[b]] = h_s[b][idx[b]]
        else:
            hsN = np.ascontiguousarray(h_s[bs])
        hsT = _maybe_round(hsN.transpose(0, 2, 1), DT_SC)
        hsN = _maybe_round(hsN, DT_CTX)
        m = {"h_tT": h_tT_c, "W": W_q, "h_sT": hsT, "h_sN": hsN}
        if use_bias:
            m["bias"] = (m_s[bs].astype(np.float32) - 1.0) * 1e11
        in_maps.append(m)
    return in_maps, idx, n, s_pad, t_pp, use_bias, use_sparse


_NC_CACHE = {}


def kernel(h_t, h_s, W_in, m_s, _run_kwargs=None):
    h_t = np.ascontiguousarray(np.asarray(h_t), dtype=np.float32)
    h_s = np.ascontiguousarray(np.asarray(h_s), dtype=np.float32)
    W_in = np.ascontiguousarray(np.asarray(W_in), dtype=np.float32)
    m_s = np.asarray(m_s)

    in_maps, idx, n, s_pad, t_pp, use_bias, use_sparse = _prepare(h_t, h_s, W_in, m_s)
    key = (s_pad, t_pp, use_bias)
    nc = _NC_CACHE.get(key)
    if nc is None:
        nc = build_kernel(s_pad, t_pp, use_bias, DT_Q, DT_SC, DT_CTX)
        nc.finalize()
        _NC_CACHE[key] = nc
    res = run_bass_kernel_spmd(
        nc, in_maps, core_ids=list(range(N_CORES)), **(_run_kwargs or {})
    )
    if _run_kwargs is not None:
        kernel.last_result = res

    context = np.empty((B, T, D), np.float32)
    scores = np.zeros((B, T, S), np.float32)
    for c in range(N_CORES):
        r = res.results[c]
        for j in range(NB):
            b = c * NB + j
            context[b] = r["context"][j]
            if use_sparse:
                scores[b][:, idx[b]] = r["probs"][j][:, : n[b]]
            else:
                scores[b] = r["probs"][j]
    return context, scores


# revision 70
# speedup vs baseline: 1.0176x; 1.0176x over previous
"""Trainium2 (8 NeuronCores) kernel for batched 'general' attention:

    energy  = h_s @ W_in^T                     [B,S,D]
    scores  = h_t @ energy^T  (masked, clamped)[B,T,S]
    probs   = softmax(scores, axis=-1)
    context = probs @ h_s                      [B,T,D]
    returns (context, probs)

Strategy:
  * Data-parallel over batch: 2 batches per core on 8 cores (no collectives).
  * Algebraic rewrite: scores = (h_t @ W_in) @ h_s^T  (saves the S*D*D matmul).
  * Mask sparsity: ~half the source positions are masked out (prob exactly 0
    in the reference).  The host compacts h_s to the unmasked columns, the
    device computes attention over the compact S, and the host scatters the
    probabilities back into the full [T,S] output (zeros elsewhere).
  * float32r (tf32-style) matmuls at 4x fp32 PE throughput; inputs are
    RNE-rounded to tf32 on the host so the f32r reads are well-defined.
  * Context is accumulated from exp(scores - max) and scaled by 1/denom at
    the PSUM->SBUF copy, so the PE transposes don't wait on normalization.
  * Dense fallback (mask mostly ones): additive -1e11 bias folded into the
    PSUM accumulation via a K=1 matmul, clamp fused into the PSUM->SBUF copy.

Self-contained: hardcodes the problem shapes; only needs the concourse repo
(/opt/trn_rl_repo) for bass.
"""

import os
import sys

import numpy as np

for _p in ("/opt/trn_rl_repo", os.path.expanduser("~/.axon_site/_ro/trn_rl_repo")):
    if os.path.isdir(_p) and _p not in sys.path:
        sys.path.insert(0, _p)

import concourse.bacc as bacc
import concourse.mybir as mybir
from concourse.bass_utils import run_bass_kernel_spmd
from concourse.masks import make_identity
from concourse.tile import TileContext, add_dep_helper

B, T, S, D = 16, 1024, 4096, 1024
N_CORES = 8
NB = B // N_CORES  # batches per core
P = 128
TI_FULL = T // P

F32 = mybir.dt.float32
F32R = mybir.dt.float32r
BF16 = mybir.dt.bfloat16


def _dt(env, default):
    v = os.environ.get(env, default)
    return {"f32": F32, "f32r": F32R, "bf16": BF16}[v]


# Matmul compute dtypes (f32r = tf32 inputs, fp32 accumulate, 4x faster).
DT_Q = _dt("ATT_DT_Q", "f32r")  # q = h_t @ W
DT_SC = _dt("ATT_DT_SC", "f32r")  # scores = q @ h_s^T
DT_CTX = _dt("ATT_DT_CTX", "bf16")  # context = probs @ h_s

MAX_SPARSE_S_PAD = 2560  # SBUF budget limit for the single-pass resident layout
SBUF_KB = 192  # per-partition budget enforced by the tile allocator


def round_tf32(x):
    """Round-to-nearest-even to a 10-bit mantissa (tf32), in float32 storage."""
    u = np.ascontiguousarray(x, np.float32).view(np.uint32)
    r = (u >> np.uint32(13)) & np.uint32(1)
    u = (u + np.uint32(0x0FFF) + r) & np.uint32(0xFFFFE000)
    return u.view(np.float32)


def _maybe_round(x, dt):
    if dt == F32R:
        return round_tf32(x)
    if dt == BF16:
        return np.ascontiguousarray(x).astype(mybir.dt.np(BF16))
    return np.ascontiguousarray(x, np.float32)


def build_kernel(s_pad, t_tiles_per_pass, use_bias, dt_q, dt_sc, dt_ctx):
    nc = bacc.Bacc(None, target_bir_lowering=False)
    KD = D // P  # contraction tiles over the feature dim
    TI = T // P  # t row-tiles
    SK = s_pad // P  # s tiles
    assert TI % t_tiles_per_pass == 0
    n_pass = TI // t_tiles_per_pass
    TB = t_tiles_per_pass
    single = n_pass == 1

    h_tT = nc.declare_dram_parameter("h_tT", [NB, D, T], dt_q, isOutput=False)
    w_in = nc.declare_dram_parameter("W", [D, D], dt_q, isOutput=False)
    h_sT = nc.declare_dram_parameter("h_sT", [NB, D, s_pad], dt_sc, isOutput=False)
    h_sN = nc.declare_dram_parameter("h_sN", [NB, s_pad, D], dt_ctx, isOutput=False)
    if use_bias:
        biasd = nc.declare_dram_parameter("bias", [NB, s_pad], F32, isOutput=False)
    probs = nc.declare_dram_parameter("probs", [NB, T, s_pad], F32, isOutput=True)
    ctx = nc.declare_dram_parameter("context", [NB, T, D], F32, isOutput=True)

    # Balanced s-chunks (all >=256 wide where possible) so no thin tail matmuls.
    n_chunks = -(-s_pad // 512)
    base = min(512, -(-(-(-s_pad // n_chunks)) // 64) * 64)
    s_chunks = []
    c = 0
    while c < s_pad:
        cw = min(base, s_pad - c)
        s_chunks.append((c, cw))
        c += cw

    # MM3 moving-operand chunk width (PSUM bank limit: 512 fp32 outputs).
    dc = 512
    hs_tile_kb = D * mybir.dt.size(dt_ctx) / 1024  # per-partition KB of one h_s tile

    # h_s-natural tiles prefetched (during phase B) vs loaded after phase B.
    if single:
        used = (
            0.5  # identity
            + KD * D * 4 / 1024  # w_sb
            + KD * T * 4 / 1024  # qT
            + TB * s_pad * 4 / 1024  # scmat
            + 2 * KD * base * 4 / 1024  # hsT chunks (bufs=2)
            + 4  # slack
        )
        a_tiles = max(0, min(SK, int((SBUF_KB - used) // hs_tile_kb)))
    else:
        a_tiles = 0

    with TileContext(nc) as tc:
        with tc.tile_pool(name="const", bufs=1) as const_pool:
            # Dependency-free warmup matmuls: get the PE sequencer into the
            # kernel body and the HAM clock-gate to full rate while the first
            # inputs stream in.
            warm = const_pool.tile([P, 512], F32)
            nc.vector.memset(warm, 0.0)
            with tc.tile_pool(name="pswarm", bufs=1, space="PSUM") as pswarm:
                wps = pswarm.tile([P, 512], F32)
                for _ in range(2):
                    nc.tensor.matmul(
                        wps[0:16, :], lhsT=warm[:, 0:16], rhs=warm, start=True, stop=True
                    )
            ident = const_pool.tile([P, P], F32)
            make_identity(nc, ident)
            use_bf_tr = single and dt_ctx == BF16
            if use_bf_tr:
                ident_bf = const_pool.tile([P, P], BF16)
                make_identity(nc, ident_bf)
            if use_bias:
                ones_t = const_pool.tile([1, P], F32)
                nc.vector.memset(ones_t, 1.0)
            w_sb = const_pool.tile([P, KD, D], dt_q)
            w_src = w_in[:, :].rearrange("(k p) d -> p k d", p=P)

            for b in range(NB):
                qT_pool = tc.alloc_tile_pool(name="qTp", bufs=1, side="right")
                if True:
                    # qT holds (h_t @ W)^T pre-rounded to the MM2 input dtype
                    qT = qT_pool.tile([P, KD, T], dt_sc)
                    # ---------------- Phase A: qT[d, t] = (h_t @ W)^T ----------------
                    with (
                        tc.tile_pool(name="phA", bufs=1) as pa,
                        tc.tile_pool(name="psA", bufs=2, space="PSUM") as psA,
                    ):
                        ht_sb = pa.tile([P, KD, T], dt_q)
                        ht_src = h_tT[b, :, :].rearrange("(k p) t -> p k t", p=P)
                        deferred = []
                        t_chunks = [(0, 128), (128, 384)] + [
                            (n0, 512) for n0 in range(512, T, 512)
                        ]
                        nc.sync.dma_start(
                            out=ht_sb[:, :, 0:128], in_=ht_src[:, :, 0:128]
                        )
                        if b == 0:
                            for j in range(KD):
                                d = nc.sync.dma_start(
                                    out=w_sb[:, :, j * P : (j + 1) * P],
                                    in_=w_src[:, :, j * P : (j + 1) * P],
                                )
                                if j >= 1:
                                    deferred.append(d)
                        for n0, nw in t_chunks[1:]:
                            deferred.append(
                                nc.sync.dma_start(
                                    out=ht_sb[:, :, n0 : n0 + nw],
                                    in_=ht_src[:, :, n0 : n0 + nw],
                                )
                            )
                        first_mm = None
                        for n0, nw in t_chunks:
                            for j in range(KD):
                                psa = psA.tile([P, 512], F32, tag="mmq")
                                for k in range(KD):
                                    mm = nc.tensor.matmul(
                                        psa[:, :nw],
                                        lhsT=w_sb[:, k, j * P : (j + 1) * P],
                                        rhs=ht_sb[:, k, n0 : n0 + nw],
                                        start=(k == 0),
                                        stop=(k == KD - 1),
                                    )
                                    if first_mm is None:
                                        first_mm = mm
                                nc.scalar.copy(
                                    out=qT[:, j, n0 : n0 + nw], in_=psa[:, :nw]
                                )
                        # Let the first matmul's inputs win the DMA bandwidth:
                        # the bulk of W/h_t loads only needs to land during MM-q.
                        for d in deferred:
                            add_dep_helper(
                                d.ins, first_mm.ins, reason="defer bulk input DMA"
                            )
                        gate_mm = first_mm

                    for ip in range(n_pass):
                        tlo = ip * TB
                        with tc.tile_pool(name="scores", bufs=1) as sc_pool:
                            scmat = sc_pool.tile([P, TB, s_pad], F32)
                            negmax = [
                                sc_pool.tile([P, 1], F32, tag=f"negmax{i}", name=f"negmax{i}")
                                for i in range(TB)
                            ]
                            denom = [
                                sc_pool.tile([P, 1], F32, tag=f"denom{i}", name=f"denom{i}")
                                for i in range(TB)
                            ]
                            recip = [
                                sc_pool.tile([P, 1], F32, tag=f"recip{i}", name=f"recip{i}")
                                for i in range(TB)
                            ]
                            den2 = [
                                sc_pool.tile([P, 1], F32, tag=f"den2{i}", name=f"den2{i}")
                                for i in range(TB)
                            ]
                            nchk = len(s_chunks)
                            pmax = [
                                sc_pool.tile([P, nchk], F32, tag=f"pmax{i}", name=f"pmax{i}")
                                for i in range(TB)
                            ]
                            with tc.tile_pool(name="hsa", bufs=1) as hsa_pool:
                                if a_tiles:
                                    hs_a = hsa_pool.tile([P, a_tiles, D], dt_ctx)
                                    for k in range(a_tiles):
                                        d = nc.sync.dma_start(
                                            out=hs_a[:, k, :],
                                            in_=h_sN[b, k * P : (k + 1) * P, :],
                                        )
                                        add_dep_helper(
                                            d.ins, gate_mm.ins, reason="defer hs_a DMA"
                                        )
                                # ---- Phase B: scores = qT^T @ h_sT (+bias), softmax ----
                                with (
                                    tc.tile_pool(name="phB", bufs=2) as pb,
                                    tc.tile_pool(name="psB", bufs=6, space="PSUM") as psB,
                                    tc.tile_pool(name="biasp", bufs=1) as bp,
                                ):
                                    if use_bias:
                                        bias_sb = bp.tile([1, s_pad], F32)
                                        nc.sync.dma_start(
                                            out=bias_sb, in_=biasd[b : b + 1, :]
                                        )
                                    for ci, (c0, cw) in enumerate(s_chunks):
                                        hsT_c = pb.tile([P, KD, base], dt_sc, tag="hsT")
                                        d = nc.sync.dma_start(
                                            out=hsT_c[:, :, :cw],
                                            in_=h_sT[b, :, c0 : c0 + cw].rearrange(
                                                "(k p) s -> p k s", p=P
                                            ),
                                        )
                                        if ci < 2:
                                            add_dep_helper(
                                                d.ins, gate_mm.ins, reason="defer hsT DMA"
                                            )
                                        for i in range(TB):
                                            ti = tlo + i
                                            psb = psB.tile([P, 512], F32, tag="mm2")
                                            if use_bias:
                                                nc.tensor.matmul(
                                                    psb[:, :cw],
                                                    lhsT=ones_t,
                                                    rhs=bias_sb[:, c0 : c0 + cw],
                                                    start=True,
                                                    stop=False,
                                                )
                                            for k in range(KD):
                                                nc.tensor.matmul(
                                                    psb[:, :cw],
                                                    lhsT=qT[:, k, ti * P : (ti + 1) * P],
                                                    rhs=hsT_c[:, k, :cw],
                                                    start=(k == 0 and not use_bias),
                                                    stop=(k == KD - 1),
                                                )
                                            if use_bias:
                                                nc.vector.tensor_scalar_max(
                                                    out=scmat[:, i, c0 : c0 + cw],
                                                    in0=psb[:, :cw],
                                                    scalar1=-1e10,
                                                )
                                            else:
                                                nc.vector.tensor_copy(
                                                    out=scmat[:, i, c0 : c0 + cw],
                                                    in_=psb[:, :cw],
                                                )
                                            nc.vector.tensor_reduce(
                                                out=pmax[i][:, ci : ci + 1],
                                                in_=scmat[:, i, c0 : c0 + cw],
                                                axis=mybir.AxisListType.X,
                                                op=mybir.AluOpType.max,
                                            )
                                # softmax: exp in place; normalization of the probs
                                # output happens after the transposes read exp().
                                sp = max(P, (SK // 2) * P)
                                for i in range(TB):
                                    nc.vector.tensor_reduce(
                                        out=negmax[i],
                                        in_=pmax[i],
                                        axis=mybir.AxisListType.X,
                                        op=mybir.AluOpType.max,
                                        negate=True,
                                    )
                                    nc.scalar.activation(
                                        out=scmat[:, i, 0:sp],
                                        in_=scmat[:, i, 0:sp],
                                        func=mybir.ActivationFunctionType.Exp,
                                        bias=negmax[i],
                                        scale=1.0,
                                        accum_out=denom[i],
                                    )
                                    if sp < s_pad:
                                        nc.scalar.activation(
                                            out=scmat[:, i, sp:s_pad],
                                            in_=scmat[:, i, sp:s_pad],
                                            func=mybir.ActivationFunctionType.Exp,
                                            bias=negmax[i],
                                            scale=1.0,
                                            accum_out=den2[i],
                                        )
                                        nc.vector.tensor_add(
                                            out=denom[i], in0=denom[i], in1=den2[i]
                                        )
                                    nc.vector.reciprocal(out=recip[i], in_=denom[i])
                                # ---------- Phase C: context = probs @ h_s ----------
                                if ip == n_pass - 1:
                                    qT_pool.release()
                                sg = max(1, SK - a_tiles) if single else 6
                                groups = [
                                    (g, min(sg, SK - g)) for g in range(a_tiles, SK, sg)
                                ]
                                with (
                                    tc.tile_pool(name="phC", bufs=(1 if single else 2)) as pc,
                                    tc.tile_pool(name="prTp", bufs=2) as prp,
                                    tc.tile_pool(name="ctxp", bufs=(2 if single else 1)) as cxp,
                                    tc.tile_pool(name="scbfp", bufs=1) as sbfp,
                                    tc.tile_pool(name="psC", bufs=3, space="PSUM") as psC,
                                    tc.tile_pool(name="psT", bufs=2, space="PSUM") as psT,
                                ):
                                    if use_bf_tr:
                                        # bf16 copy of exp(scores): prT is bf16
                                        # anyway, so rounding before the PE
                                        # transpose halves its cycle cost with
                                        # bit-identical MM3 inputs.
                                        scbf = sbfp.tile([P, TB, s_pad], BF16)
                                    if not single:
                                        ctx_acc = [
                                            cxp.tile(
                                                [P, D],
                                                F32,
                                                tag=f"ctxacc{i}",
                                                name=f"ctxacc{i}",
                                            )
                                            for i in range(TB)
                                        ]
                                    hs_bt = {}

                                    def load_group(g0, gw):
                                        hs_gr = pc.tile(
                                            [P, sg, D], dt_ctx, tag="hsgr", name="hsgr"
                                        )
                                        for k0 in range(0, gw, 5):
                                            kw = min(5, gw - k0)
                                            nc.sync.dma_start(
                                                out=hs_gr[:, k0 : k0 + kw, :],
                                                in_=h_sN[
                                                    b,
                                                    (g0 + k0) * P : (g0 + k0 + kw) * P,
                                                    :,
                                                ].rearrange("(k p) d -> p k d", p=P),
                                            )
                                        for k in range(gw):
                                            hs_bt[g0 + k] = hs_gr[:, k, :]

                                    if single:
                                        for g0, gw in groups:
                                            load_group(g0, gw)
                                        groups = [(0, SK)]
                                        if a_tiles:
                                            for k in range(a_tiles):
                                                hs_bt[k] = hs_a[:, k, :]
                                    for g0, gw in groups:
                                        if not single:
                                            load_group(g0, gw)
                                        for i in range(TB):
                                            prT = prp.tile(
                                                [P, sg if not single else SK, P],
                                                dt_ctx,
                                                tag="prT",
                                                name="prT",
                                            )
                                            if use_bf_tr:
                                                # split at the exp-half boundary
                                                # so the first transposes start
                                                # while the second exp half runs
                                                nc.vector.tensor_copy(
                                                    out=scbf[:, i, 0:sp],
                                                    in_=scmat[:, i, 0:sp],
                                                )
                                                if sp < s_pad:
                                                    nc.vector.tensor_copy(
                                                        out=scbf[:, i, sp:s_pad],
                                                        in_=scmat[:, i, sp:s_pad],
                                                    )
                                            for k in range(gw):
                                                if use_bf_tr:
                                                    pst = psT.tile(
                                                        [P, P], BF16, tag="tr", name="pst"
                                                    )
                                                    nc.tensor.transpose(
                                                        pst,
                                                        in_=scbf[
                                                            :,
                                                            i,
                                                            (g0 + k) * P : (g0 + k + 1) * P,
                                                        ],
                                                        identity=ident_bf,
                                                    )
                                                else:
                                                    pst = psT.tile(
                                                        [P, P], F32, tag="tr", name="pst"
                                                    )
                                                    nc.tensor.transpose(
                                                        pst,
                                                        in_=scmat[
                                                            :,
                                                            i,
                                                            (g0 + k) * P : (g0 + k + 1) * P,
                                                        ],
                                                        identity=ident,
                                                    )
                                                nc.vector.tensor_copy(
                                                    out=prT[:, k, :], in_=pst
                                                )
                                            for d0 in range(0, D, dc):
                                                psc = psC.tile(
                                                    [P, dc], F32, tag="mm3", name="psc"
                                                )
                                                for k in range(gw):
                                                    nc.tensor.matmul(
                                                        psc,
                                                        lhsT=prT[:, k, :],
                                                        rhs=hs_bt[g0 + k][:, d0 : d0 + dc],
                                                        start=(k == 0),
                                                        stop=(k == gw - 1),
                                                    )
                                                if single:
                                                    if d0 == 0:
                                                        ctx_sb = cxp.tile(
                                                            [P, D],
                                                            F32,
                                                            tag="ctx",
                                                            name="ctx_sb",
                                                        )
                                                    nc.scalar.mul(
                                                        out=ctx_sb[:, d0 : d0 + dc],
                                                        in_=psc,
                                                        mul=recip[i],
                                                    )
                                                elif g0 == a_tiles:
                                                    nc.scalar.mul(
                                                        out=ctx_acc[i][:, d0 : d0 + dc],
                                                        in_=psc,
                                                        mul=recip[i],
                                                    )
                                                else:
                                                    nc.vector.scalar_tensor_tensor(
                                                        out=ctx_acc[i][:, d0 : d0 + dc],
                                                        in0=psc,
                                                        scalar=recip[i],
                                                        in1=ctx_acc[i][:, d0 : d0 + dc],
                                                        op0=mybir.AluOpType.mult,
                                                        op1=mybir.AluOpType.add,
                                                    )
                                            if single:
                                                nc.sync.dma_start(
                                                    out=ctx[
                                                        b,
                                                        (tlo + i) * P : (tlo + i + 1) * P,
                                                        :,
                                                    ],
                                                    in_=ctx_sb,
                                                )
                                                # normalize probs for row-tile i
                                                # right after its transposes read
                                                # the unnormalized exp values;
                                                # on ScalarE so the DVE cast
                                                # chain feeding MM3 stays clear
                                                nc.scalar.mul(
                                                    out=scmat[:, i, :],
                                                    in_=scmat[:, i, :],
                                                    mul=recip[i],
                                                )
                                                nc.sync.dma_start(
                                                    out=probs[
                                                        b,
                                                        (tlo + i) * P : (tlo + i + 1) * P,
                                                        :,
                                                    ],
                                                    in_=scmat[:, i, :],
                                                )
                                    if not single:
                                        for i in range(TB):
                                            nc.vector.tensor_scalar_mul(
                                                out=scmat[:, i, :],
                                                in0=scmat[:, i, :],
                                                scalar1=recip[i],
                                            )
                                            nc.sync.dma_start(
                                                out=probs[
                                                    b, (tlo + i) * P : (tlo + i + 1) * P, :
                                                ],
                                                in_=scmat[:, i, :],
                                            )
                                        for i in range(TB):
                                            nc.sync.dma_start(
                                                out=ctx[
                                                    b, (tlo + i) * P : (tlo + i + 1) * P, :
                                                ],
                                                in_=ctx_acc[i],
                                            )
    return nc


def _prepare(h_t, h_s, W_in, m_s):
    """Choose sparse/dense layout and build per-core input maps."""
    idx = [np.flatnonzero(m_s[b]) for b in range(B)]
    n = [len(ix) for ix in idx]
    n_max = max(n) if n else 0
    s_pad = max(P, ((n_max + P - 1) // P) * P)
    use_sparse = (
        s_pad <= 3584 and min(n) > 0 and os.environ.get("ATT_FORCE_DENSE") != "1"
    )
    if use_sparse:
        t_pp, use_bias = (TI_FULL if s_pad <= MAX_SPARSE_S_PAD else 4), False
    else:
        s_pad, t_pp, use_bias = S, 2, True

    W_q = _maybe_round(W_in, DT_Q)
    in_maps = []
    for c in range(N_CORES):
        bs = list(range(c * NB, (c + 1) * NB))
        h_tT_c = _maybe_round(h_t[bs].transpose(0, 2, 1), DT_Q)
        if use_sparse:
            hsN = np.zeros((NB, s_pad, D), np.float32)
            for j, b in enumerate(bs):
                hsN[j, : n[b]] = h_s[b][idx[b]]
        else:
            hsN = np.ascontiguousarray(h_s[bs])
        hsT = _maybe_round(hsN.transpose(0, 2, 1), DT_SC)
        hsN = _maybe_round(hsN, DT_CTX)
        m = {"h_tT": h_tT_c, "W": W_q, "h_sT": hsT, "h_sN": hsN}
        if use_bias:
            m["bias"] = (m_s[bs].astype(np.float32) - 1.0) * 1e11
        in_maps.append(m)
    return in_maps, idx, n, s_pad, t_pp, use_bias, use_sparse


_NC_CACHE = {}


def kernel(h_t, h_s, W_in, m_s, _run_kwargs=None):
    h_t = np.ascontiguousarray(np.asarray(h_t), dtype=np.float32)
    h_s = np.ascontiguousarray(np.asarray(h_s), dtype=np.float32)
    W_in = np.ascontiguousarray(np.asarray(W_in), dtype=np.float32)
    m_s = np.asarray(m_s)

    in_maps, idx, n, s_pad, t_pp, use_bias, use_sparse = _prepare(h_t, h_s, W_in, m_s)
    key = (s_pad, t_pp, use_bias)
    nc = _NC_CACHE.get(key)
    if nc is None:
        nc = build_kernel(s_pad, t_pp, use_bias, DT_Q, DT_SC, DT_CTX)
        nc.finalize()
        _NC_CACHE[key] = nc
    res = run_bass_kernel_spmd(
        nc, in_maps, core_ids=list(range(N_CORES)), **(_run_kwargs or {})
    )
    if _run_kwargs is not None:
        kernel.last_result = res

    context = np.empty((B, T, D), np.float32)
    scores = np.zeros((B, T, S), np.float32)
    for c in range(N_CORES):
        r = res.results[c]
        for j in range(NB):
            b = c * NB + j
            context[b] = r["context"][j]
            if use_sparse:
                scores[b][:, idx[b]] = r["probs"][j][:, : n[b]]
            else:
                scores[b] = r["probs"][j]
    return context, scores


# revision 72
# speedup vs baseline: 1.0573x; 1.0390x over previous
"""Trainium2 (8 NeuronCores) kernel for batched 'general' attention:

    energy  = h_s @ W_in^T                     [B,S,D]
    scores  = h_t @ energy^T  (masked, clamped)[B,T,S]
    probs   = softmax(scores, axis=-1)
    context = probs @ h_s                      [B,T,D]
    returns (context, probs)

Strategy:
  * Data-parallel over batch: 2 batches per core on 8 cores (no collectives).
  * Algebraic rewrite: scores = (h_t @ W_in) @ h_s^T  (saves the S*D*D matmul).
  * Mask sparsity: ~half the source positions are masked out (prob exactly 0
    in the reference).  The host compacts h_s to the unmasked columns, the
    device computes attention over the compact S, and the host scatters the
    probabilities back into the full [T,S] output (zeros elsewhere).
  * float32r (tf32-style) matmuls at 4x fp32 PE throughput; inputs are
    RNE-rounded to tf32 on the host so the f32r reads are well-defined.
  * Context is accumulated from exp(scores - max) and scaled by 1/denom at
    the PSUM->SBUF copy, so the PE transposes don't wait on normalization.
  * Dense fallback (mask mostly ones): additive -1e11 bias folded into the
    PSUM accumulation via a K=1 matmul, clamp fused into the PSUM->SBUF copy.

Self-contained: hardcodes the problem shapes; only needs the concourse repo
(/opt/trn_rl_repo) for bass.
"""

import os
import sys

import numpy as np

for _p in ("/opt/trn_rl_repo", os.path.expanduser("~/.axon_site/_ro/trn_rl_repo")):
    if os.path.isdir(_p) and _p not in sys.path:
        sys.path.insert(0, _p)

import concourse.bacc as bacc
import concourse.mybir as mybir
from concourse.bass_utils import run_bass_kernel_spmd
from concourse.masks import make_identity
from concourse.tile import TileContext, add_dep_helper

B, T, S, D = 16, 1024, 4096, 1024
N_CORES = 8
NB = B // N_CORES  # batches per core
P = 128
TI_FULL = T // P

F32 = mybir.dt.float32
F32R = mybir.dt.float32r
BF16 = mybir.dt.bfloat16


def _dt(env, default):
    v = os.environ.get(env, default)
    return {"f32": F32, "f32r": F32R, "bf16": BF16}[v]


# Matmul compute dtypes (f32r = tf32 inputs, fp32 accumulate, 4x faster).
DT_Q = _dt("ATT_DT_Q", "f32r")  # q = h_t @ W
DT_SC = _dt("ATT_DT_SC", "f32r")  # scores = q @ h_s^T
DT_CTX = _dt("ATT_DT_CTX", "bf16")  # context = probs @ h_s

MAX_SPARSE_S_PAD = 2560  # SBUF budget limit for the single-pass resident layout
SBUF_KB = 192  # per-partition budget enforced by the tile allocator


def round_tf32(x):
    """Round-to-nearest-even to a 10-bit mantissa (tf32), in float32 storage."""
    u = np.ascontiguousarray(x, np.float32).view(np.uint32)
    r = (u >> np.uint32(13)) & np.uint32(1)
    u = (u + np.uint32(0x0FFF) + r) & np.uint32(0xFFFFE000)
    return u.view(np.float32)


def _maybe_round(x, dt):
    if dt == F32R:
        return round_tf32(x)
    if dt == BF16:
        return np.ascontiguousarray(x).astype(mybir.dt.np(BF16))
    return np.ascontiguousarray(x, np.float32)


def build_kernel(s_pad, t_tiles_per_pass, use_bias, dt_q, dt_sc, dt_ctx):
    nc = bacc.Bacc(None, target_bir_lowering=False)
    KD = D // P  # contraction tiles over the feature dim
    TI = T // P  # t row-tiles
    SK = s_pad // P  # s tiles
    assert TI % t_tiles_per_pass == 0
    n_pass = TI // t_tiles_per_pass
    TB = t_tiles_per_pass
    single = n_pass == 1

    h_tT = nc.declare_dram_parameter("h_tT", [NB, D, T], dt_q, isOutput=False)
    w_in = nc.declare_dram_parameter("W", [D, D], dt_q, isOutput=False)
    h_sT = nc.declare_dram_parameter("h_sT", [NB, D, s_pad], dt_sc, isOutput=False)
    h_sN = nc.declare_dram_parameter("h_sN", [NB, s_pad, D], dt_ctx, isOutput=False)
    if use_bias:
        biasd = nc.declare_dram_parameter("bias", [NB, s_pad], F32, isOutput=False)
    probs = nc.declare_dram_parameter("probs", [NB, T, s_pad], F32, isOutput=True)
    ctx = nc.declare_dram_parameter("context", [NB, T, D], F32, isOutput=True)

    # Balanced s-chunks (all >=256 wide where possible) so no thin tail matmuls.
    n_chunks = -(-s_pad // 512)
    base = min(512, -(-(-(-s_pad // n_chunks)) // 64) * 64)
    s_chunks = []
    c = 0
    while c < s_pad:
        cw = min(base, s_pad - c)
        s_chunks.append((c, cw))
        c += cw

    # MM3 moving-operand chunk width (PSUM bank limit: 512 fp32 outputs).
    dc = 512
    hs_tile_kb = D * mybir.dt.size(dt_ctx) / 1024  # per-partition KB of one h_s tile

    # h_s-natural tiles prefetched (during phase B) vs loaded after phase B.
    if single:
        used = (
            0.5  # identity
            + KD * D * 4 / 1024  # w_sb
            + KD * T * 4 / 1024  # qT
            + TB * s_pad * 4 / 1024  # scmat
            + 2 * KD * base * 4 / 1024  # hsT chunks (bufs=2)
            + 4  # slack
        )
        a_tiles = max(0, min(SK, int((SBUF_KB - used) // hs_tile_kb)))
    else:
        a_tiles = 0

    with TileContext(nc) as tc:
        with tc.tile_pool(name="const", bufs=1) as const_pool:
            # Dependency-free warmup matmuls: get the PE sequencer into the
            # kernel body and the HAM clock-gate to full rate while the first
            # inputs stream in.
            warm = const_pool.tile([P, 512], F32)
            nc.vector.memset(warm, 0.0)
            with tc.tile_pool(name="pswarm", bufs=1, space="PSUM") as pswarm:
                wps = pswarm.tile([P, 512], F32)
                for _ in range(2):
                    nc.tensor.matmul(
                        wps[0:16, :], lhsT=warm[:, 0:16], rhs=warm, start=True, stop=True
                    )
            ident = const_pool.tile([P, P], F32)
            make_identity(nc, ident)
            use_bf_tr = single and dt_ctx == BF16
            if use_bf_tr:
                ident_bf = const_pool.tile([P, P], BF16)
                make_identity(nc, ident_bf)
            if use_bias:
                ones_t = const_pool.tile([1, P], F32)
                nc.vector.memset(ones_t, 1.0)
            w_sb = const_pool.tile([P, KD, D], dt_q)
            w_src = w_in[:, :].rearrange("(k p) d -> p k d", p=P)

            for b in range(NB):
                qT_pool = tc.alloc_tile_pool(name="qTp", bufs=1, side="right")
                if True:
                    # qT holds (h_t @ W)^T pre-rounded to the MM2 input dtype
                    qT = qT_pool.tile([P, KD, T], dt_sc)
                    # ---------------- Phase A: qT[d, t] = (h_t @ W)^T ----------------
                    with (
                        tc.tile_pool(name="phA", bufs=1) as pa,
                        tc.tile_pool(name="psA", bufs=2, space="PSUM") as psA,
                    ):
                        ht_sb = pa.tile([P, KD, T], dt_q)
                        ht_src = h_tT[b, :, :].rearrange("(k p) t -> p k t", p=P)
                        deferred = []
                        t_chunks = [(0, 256), (256, 256)] + [
                            (n0, 512) for n0 in range(512, T, 512)
                        ]
                        nc.sync.dma_start(
                            out=ht_sb[:, :, 0:256], in_=ht_src[:, :, 0:256]
                        )
                        if b == 0:
                            for j in range(KD):
                                d = nc.sync.dma_start(
                                    out=w_sb[:, :, j * P : (j + 1) * P],
                                    in_=w_src[:, :, j * P : (j + 1) * P],
                                )
                                if j >= 1:
                                    deferred.append(d)
                        for n0, nw in t_chunks[1:]:
                            deferred.append(
                                nc.sync.dma_start(
                                    out=ht_sb[:, :, n0 : n0 + nw],
                                    in_=ht_src[:, :, n0 : n0 + nw],
                                )
                            )
                        first_mm = None
                        for n0, nw in t_chunks:
                            for j in range(KD):
                                psa = psA.tile([P, 512], F32, tag="mmq")
                                for k in range(KD):
                                    mm = nc.tensor.matmul(
                                        psa[:, :nw],
                                        lhsT=w_sb[:, k, j * P : (j + 1) * P],
                                        rhs=ht_sb[:, k, n0 : n0 + nw],
                                        start=(k == 0),
                                        stop=(k == KD - 1),
                                    )
                                    if first_mm is None:
                                        first_mm = mm
                                nc.scalar.copy(
                                    out=qT[:, j, n0 : n0 + nw], in_=psa[:, :nw]
                                )
                        # Let the first matmul's inputs win the DMA bandwidth:
                        # the bulk of W/h_t loads only needs to land during MM-q.
                        for d in deferred:
                            add_dep_helper(
                                d.ins, first_mm.ins, reason="defer bulk input DMA"
                            )
                        gate_mm = first_mm

                    for ip in range(n_pass):
                        tlo = ip * TB
                        with tc.tile_pool(name="scores", bufs=1) as sc_pool:
                            scmat = sc_pool.tile([P, TB, s_pad], F32)
                            negmax = [
                                sc_pool.tile([P, 1], F32, tag=f"negmax{i}", name=f"negmax{i}")
                                for i in range(TB)
                            ]
                            denom = [
                                sc_pool.tile([P, 1], F32, tag=f"denom{i}", name=f"denom{i}")
                                for i in range(TB)
                            ]
                            recip = [
                                sc_pool.tile([P, 1], F32, tag=f"recip{i}", name=f"recip{i}")
                                for i in range(TB)
                            ]
                            den2 = [
                                sc_pool.tile([P, 1], F32, tag=f"den2{i}", name=f"den2{i}")
                                for i in range(TB)
                            ]
                            nchk = len(s_chunks)
                            pmax = [
                                sc_pool.tile([P, nchk], F32, tag=f"pmax{i}", name=f"pmax{i}")
                                for i in range(TB)
                            ]
                            with tc.tile_pool(name="hsa", bufs=1) as hsa_pool:
                                if a_tiles:
                                    hs_a = hsa_pool.tile([P, a_tiles, D], dt_ctx)
                                    for k in range(a_tiles):
                                        d = nc.sync.dma_start(
                                            out=hs_a[:, k, :],
                                            in_=h_sN[b, k * P : (k + 1) * P, :],
                                        )
                                        add_dep_helper(
                                            d.ins, gate_mm.ins, reason="defer hs_a DMA"
                                        )
                                # ---- Phase B: scores = qT^T @ h_sT (+bias), softmax ----
                                with (
                                    tc.tile_pool(name="phB", bufs=2) as pb,
                                    tc.tile_pool(name="psB", bufs=6, space="PSUM") as psB,
                                    tc.tile_pool(name="biasp", bufs=1) as bp,
                                ):
                                    if use_bias:
                                        bias_sb = bp.tile([1, s_pad], F32)
                                        nc.sync.dma_start(
                                            out=bias_sb, in_=biasd[b : b + 1, :]
                                        )
                                    for ci, (c0, cw) in enumerate(s_chunks):
                                        hsT_c = pb.tile([P, KD, base], dt_sc, tag="hsT")
                                        d = nc.sync.dma_start(
                                            out=hsT_c[:, :, :cw],
                                            in_=h_sT[b, :, c0 : c0 + cw].rearrange(
                                                "(k p) s -> p k s", p=P
                                            ),
                                        )
                                        if ci < 2:
                                            add_dep_helper(
                                                d.ins, gate_mm.ins, reason="defer hsT DMA"
                                            )
                                        for i in range(TB):
                                            ti = tlo + i
                                            psb = psB.tile([P, 512], F32, tag="mm2")
                                            if use_bias:
                                                nc.tensor.matmul(
                                                    psb[:, :cw],
                                                    lhsT=ones_t,
                                                    rhs=bias_sb[:, c0 : c0 + cw],
                                                    start=True,
                                                    stop=False,
                                                )
                                            for k in range(KD):
                                                nc.tensor.matmul(
                                                    psb[:, :cw],
                                                    lhsT=qT[:, k, ti * P : (ti + 1) * P],
                                                    rhs=hsT_c[:, k, :cw],
                                                    start=(k == 0 and not use_bias),
                                                    stop=(k == KD - 1),
                                                )
                                            if use_bias:
                                                nc.vector.tensor_scalar_max(
                                                    out=scmat[:, i, c0 : c0 + cw],
                                                    in0=psb[:, :cw],
                                                    scalar1=-1e10,
                                                )
                                            else:
                                                nc.vector.tensor_copy(
                                                    out=scmat[:, i, c0 : c0 + cw],
                                                    in_=psb[:, :cw],
                                                )
                                            nc.vector.tensor_reduce(
                                                out=pmax[i][:, ci : ci + 1],
                                                in_=scmat[:, i, c0 : c0 + cw],
                                                axis=mybir.AxisListType.X,
                                                op=mybir.AluOpType.max,
                                            )
                                # softmax: exp in place; normalization of the probs
                                # output happens after the transposes read exp().
                                sp = max(P, (SK // 2) * P)
                                for i in range(TB):
                                    nc.vector.tensor_reduce(
                                        out=negmax[i],
                                        in_=pmax[i],
                                        axis=mybir.AxisListType.X,
                                        op=mybir.AluOpType.max,
                                        negate=True,
                                    )
                                    nc.scalar.activation(
                                        out=scmat[:, i, 0:sp],
                                        in_=scmat[:, i, 0:sp],
                                        func=mybir.ActivationFunctionType.Exp,
                                        bias=negmax[i],
                                        scale=1.0,
                                        accum_out=denom[i],
                                    )
                                    if sp < s_pad:
                                        nc.scalar.activation(
                                            out=scmat[:, i, sp:s_pad],
                                            in_=scmat[:, i, sp:s_pad],
                                            func=mybir.ActivationFunctionType.Exp,
                                            bias=negmax[i],
                                            scale=1.0,
                                            accum_out=den2[i],
                                        )
                                        nc.vector.tensor_add(
                                            out=denom[i], in0=denom[i], in1=den2[i]
                                        )
                                    nc.vector.reciprocal(out=recip[i], in_=denom[i])
                                # ---------- Phase C: context = probs @ h_s ----------
                                if ip == n_pass - 1:
                                    qT_pool.release()
                                sg = max(1, SK - a_tiles) if single else 6
                                groups = [
                                    (g, min(sg, SK - g)) for g in range(a_tiles, SK, sg)
                                ]
                                with (
                                    tc.tile_pool(name="phC", bufs=(1 if single else 2)) as pc,
                                    tc.tile_pool(name="prTp", bufs=3) as prp,
                                    tc.tile_pool(name="ctxp", bufs=(2 if single else 1)) as cxp,
                                    tc.tile_pool(name="scbfp", bufs=1) as sbfp,
                                    tc.tile_pool(name="psC", bufs=3, space="PSUM") as psC,
                                    tc.tile_pool(name="psT", bufs=3, space="PSUM") as psT,
                                ):
                                    if use_bf_tr:
                                        # bf16 copy of exp(scores): prT is bf16
                                        # anyway, so rounding before the PE
                                        # transpose halves its cycle cost with
                                        # bit-identical MM3 inputs.
                                        scbf = sbfp.tile([P, TB, s_pad], BF16)
                                    if not single:
                                        ctx_acc = [
                                            cxp.tile(
                                                [P, D],
                                                F32,
                                                tag=f"ctxacc{i}",
                                                name=f"ctxacc{i}",
                                            )
                                            for i in range(TB)
                                        ]
                                    hs_bt = {}

                                    def load_group(g0, gw):
                                        hs_gr = pc.tile(
                                            [P, sg, D], dt_ctx, tag="hsgr", name="hsgr"
                                        )
                                        for k0 in range(0, gw, 5):
                                            kw = min(5, gw - k0)
                                            nc.sync.dma_start(
                                                out=hs_gr[:, k0 : k0 + kw, :],
                                                in_=h_sN[
                                                    b,
                                                    (g0 + k0) * P : (g0 + k0 + kw) * P,
                                                    :,
                                                ].rearrange("(k p) d -> p k d", p=P),
                                            )
                                        for k in range(gw):
                                            hs_bt[g0 + k] = hs_gr[:, k, :]

                                    if single:
                                        for g0, gw in groups:
                                            load_group(g0, gw)
                                        groups = [(0, SK)]
                                        if a_tiles:
                                            for k in range(a_tiles):
                                                hs_bt[k] = hs_a[:, k, :]
                                    for g0, gw in groups:
                                        if not single:
                                            load_group(g0, gw)
                                        for i in range(TB):
                                            prT = prp.tile(
                                                [P, sg if not single else SK, P],
                                                dt_ctx,
                                                tag="prT",
                                                name="prT",
                                            )
                                            if use_bf_tr:
                                                # split at the exp-half boundary
                                                # so the first transposes start
                                                # while the second exp half runs
                                                nc.vector.tensor_copy(
                                                    out=scbf[:, i, 0:sp],
                                                    in_=scmat[:, i, 0:sp],
                                                )
                                                if sp < s_pad:
                                                    nc.vector.tensor_copy(
                                                        out=scbf[:, i, sp:s_pad],
                                                        in_=scmat[:, i, sp:s_pad],
                                                    )
                                            for k in range(gw):
                                                if use_bf_tr:
                                                    pst = psT.tile(
                                                        [P, P], BF16, tag="tr", name="pst"
                                                    )
                                                    nc.tensor.transpose(
                                                        pst,
                                                        in_=scbf[
                                                            :,
                                                            i,
                                                            (g0 + k) * P : (g0 + k + 1) * P,
                                                        ],
                                                        identity=ident_bf,
                                                    )
                                                else:
                                                    pst = psT.tile(
                                                        [P, P], F32, tag="tr", name="pst"
                                                    )
                                                    nc.tensor.transpose(
                                                        pst,
                                                        in_=scmat[
                                                            :,
                                                            i,
                                                            (g0 + k) * P : (g0 + k + 1) * P,
                                                        ],
                                                        identity=ident,
                                                    )
                                                nc.vector.tensor_copy(
                                                    out=prT[:, k, :], in_=pst
                                                )
                                            for d0 in range(0, D, dc):
                                                psc = psC.tile(
                                                    [P, dc], F32, tag="mm3", name="psc"
                                                )
                                                for k in range(gw):
                                                    nc.tensor.matmul(
                                                        psc,
                                                        lhsT=prT[:, k, :],
                                                        rhs=hs_bt[g0 + k][:, d0 : d0 + dc],
                                                        start=(k == 0),
                                                        stop=(k == gw - 1),
                                                    )
                                                if single:
                                                    if d0 == 0:
                                                        ctx_sb = cxp.tile(
                                                            [P, D],
                                                            F32,
                                                            tag="ctx",
                                                            name="ctx_sb",
                                                        )
                                                    nc.scalar.mul(
                                                        out=ctx_sb[:, d0 : d0 + dc],
                                                        in_=psc,
                                                        mul=recip[i],
                                                    )
                                                elif g0 == a_tiles:
                                                    nc.scalar.mul(
                                                        out=ctx_acc[i][:, d0 : d0 + dc],
                                                        in_=psc,
                                                        mul=recip[i],
                                                    )
                                                else:
                                                    nc.vector.scalar_tensor_tensor(
                                                        out=ctx_acc[i][:, d0 : d0 + dc],
                                                        in0=psc,
                                                        scalar=recip[i],
                                                        in1=ctx_acc[i][:, d0 : d0 + dc],
                                                        op0=mybir.AluOpType.mult,
                                                        op1=mybir.AluOpType.add,
                                                    )
                                            if single:
                                                nc.sync.dma_start(
                                                    out=ctx[
                                                        b,
                                                        (tlo + i) * P : (tlo + i + 1) * P,
                                                        :,
                                                    ],
                                                    in_=ctx_sb,
                                                )
                                                # normalize probs for row-tile i
                                                # right after its transposes read
                                                # the unnormalized exp values;
                                                # on ScalarE so the DVE cast
                                                # chain feeding MM3 stays clear
                                                nc.scalar.mul(
                                                    out=scmat[:, i, :],
                                                    in_=scmat[:, i, :],
                                                    mul=recip[i],
                                                )
                                                nc.sync.dma_start(
                                                    out=probs[
                                                        b,
                                                        (tlo + i) * P : (tlo + i + 1) * P,
                                                        :,
                                                    ],
                                                    in_=scmat[:, i, :],
                                                )
                                    if not single:
                                        for i in range(TB):
                                            nc.vector.tensor_scalar_mul(
                                                out=scmat[:, i, :],
                                                in0=scmat[:, i, :],
                                                scalar1=recip[i],
                                            )
                                            nc.sync.dma_start(
                                                out=probs[
                                                    b, (tlo + i) * P : (tlo + i + 1) * P, :
                                                ],
                                                in_=scmat[:, i, :],
                                            )
                                        for i in range(TB):
                                            nc.sync.dma_start(
                                                out=ctx[
                                                    b, (tlo + i) * P : (tlo + i + 1) * P, :
                                                ],
                                                in_=ctx_acc[i],
                                            )
    return nc


def _prepare(h_t, h_s, W_in, m_s):
    """Choose sparse/dense layout and build per-core input maps."""
    idx = [np.flatnonzero(m_s[b]) for b in range(B)]
    n = [len(ix) for ix in idx]
    n_max = max(n) if n else 0
    s_pad = max(P, ((n_max + P - 1) // P) * P)
    use_sparse = (
        s_pad <= 3584 and min(n) > 0 and os.environ.get("ATT_FORCE_DENSE") != "1"
    )
    if use_sparse:
        t_pp, use_bias = (TI_FULL if s_pad <= MAX_SPARSE_S_PAD else 4), False
    else:
        s_pad, t_pp, use_bias = S, 2, True

    W_q = _maybe_round(W_in, DT_Q)
    in_maps = []
    for c in range(N_CORES):
        bs = list(range(c * NB, (c + 1) * NB))
        h_tT_c = _maybe_round(h_t[bs].transpose(0, 2, 1), DT_Q)
        if use_sparse:
            hsN = np.zeros((NB, s_pad, D), np.float32)
            for j, b in enumerate(bs):
                hsN[j, : n[b]] = h_s[b][idx[b]]
        else:
            hsN = np.ascontiguousarray(h_s[bs])
        hsT = _maybe_round(hsN.transpose(0, 2, 1), DT_SC)
        hsN = _maybe_round(hsN, DT_CTX)
        m = {"h_tT": h_tT_c, "W": W_q, "h_sT": hsT, "h_sN": hsN}
        if use_bias:
            m["bias"] = (m_s[bs].astype(np.float32) - 1.0) * 1e11
        in_maps.append(m)
    return in_maps, idx, n, s_pad, t_pp, use_bias, use_sparse


_NC_CACHE = {}


def kernel(h_t, h_s, W_in, m_s, _run_kwargs=None):
    h_t = np.ascontiguousarray(np.asarray(h_t), dtype=np.float32)
    h_s = np.ascontiguousarray(np.asarray(h_s), dtype=np.float32)
    W_in = np.ascontiguousarray(np.asarray(W_in), dtype=np.float32)
    m_s = np.asarray(m_s)

    in_maps, idx, n, s_pad, t_pp, use_bias, use_sparse = _prepare(h_t, h_s, W_in, m_s)
    key = (s_pad, t_pp, use_bias)
    nc = _NC_CACHE.get(key)
    if nc is None:
        nc = build_kernel(s_pad, t_pp, use_bias, DT_Q, DT_SC, DT_CTX)
        nc.finalize()
        _NC_CACHE[key] = nc
    res = run_bass_kernel_spmd(
        nc, in_maps, core_ids=list(range(N_CORES)), **(_run_kwargs or {})
    )
    if _run_kwargs is not None:
        kernel.last_result = res

    context = np.empty((B, T, D), np.float32)
    scores = np.zeros((B, T, S), np.float32)
    for c in range(N_CORES):
        r = res.results[c]
        for j in range(NB):
            b = c * NB + j
            context[b] = r["context"][j]
            if use_sparse:
                scores[b][:, idx[b]] = r["probs"][j][:, : n[b]]
            else:
                scores[b] = r["probs"][j]
    return context, scores
